# revision 1
# baseline (speedup 1.0000x reference)
"""Distributed Trainium2 kernel for the dense-graph GNN layer.

Math: with xn = x/||x|| (rows), G = xn@xn.T, d = rsqrt(G@1),
out = (diag(d) G diag(d) x) W.  The N x N Gram matrix is never needed:
  G @ 1        = xn @ t,            t = colsum(xn)            [D]
  diag(d) G diag(d) x = f * (x @ z),  z = x.T @ diag(f) @ x   [D, D]
  f_i = d_i / ||x_i||   (combines both scalings; z is symmetric)
  out = f * (x @ (z @ W))
So each core processes its 1024-row shard with O(N D^2) flops and the only
cross-core traffic is an AllGather of a [1,256] colsum partial and an
AllReduce of the [256,256] (z @ W) partial.
"""

import os
import sys

import numpy as np

for _p in ("/opt/trn_rl_repo", "/root/.axon_site/_ro/trn_rl_repo"):
    if os.path.isdir(_p) and _p not in sys.path:
        sys.path.insert(0, _p)

import concourse.bacc as bacc
import concourse.mybir as mybir
import concourse.tile as tile
import concourse.masks as masks
from concourse import bass_utils

R = 8                 # cores
N, D = 8192, 256
NL = N // R           # 1024 rows per core
P = 128
T = NL // P           # 8 row tiles per core
F32 = mybir.dt.float32
BF16 = mybir.dt.bfloat16
AF = mybir.ActivationFunctionType
ALU = mybir.AluOpType

_cache = {}


def _program(tc, x, W, out):
    nc = tc.nc
    rg = [list(range(R))]
    if True:
        with (
            tc.tile_pool(name="persist", bufs=1) as pp,
            tc.tile_pool(name="work", bufs=3) as wp,
            tc.tile_pool(name="psum", bufs=1, space="PSUM") as psp,
            tc.tile_pool(name="psumw", bufs=4, space="PSUM") as psw,
            tc.tile_pool(name="dram", bufs=1, space="DRAM") as dp,
        ):
            x_all = pp.tile([P, T * D], F32)      # row tile i at [:, i*D:(i+1)*D]
            xb_all = pp.tile([P, T * D], BF16)    # bf16 copy of x
            g_all = pp.tile([P, T * D], BF16)     # f * x (bf16)
            xT_all = pp.tile([P, 2 * NL], BF16)   # x.T chunk c at [:, c*NL + i*P]
            W_sb = pp.tile([P, 2 * D], F32)       # W k-chunk kc at [:, kc*D]
            Wb_sb = pp.tile([P, 2 * D], BF16)
            zw_sb = pp.tile([P, 2 * D], BF16)     # zw a-chunk ka at [:, ka*D]
            zT_sb = pp.tile([P, 2 * D], BF16)

            ss = pp.tile([P, T], F32)
            invn = pp.tile([P, T], F32)
            nrm = pp.tile([P, T], F32)
            stl = pp.tile([P, T], F32)
            s_t = pp.tile([P, T], F32)
            sq_s = pp.tile([P, T], F32)
            dd = pp.tile([P, T], F32)
            f_t = pp.tile([P, T], F32)

            ident = pp.tile([P, P], F32)
            masks.make_identity(nc, ident[:])
            ones8 = pp.tile([8, P], F32)
            nc.gpsimd.memset(ones8[:], 1.0)

            cc_t_in = dp.tile([1, D], F32)
            cc_t_out = dp.tile([R, D], F32)
            cc_zw_in = dp.tile([2 * P, D], BF16)
            cc_zw_out = dp.tile([2 * P, D], BF16)

            for kc in range(2):
                nc.sync.dma_start(W_sb[:, kc * D:(kc + 1) * D], W[kc * P:(kc + 1) * P, :])
            nc.vector.tensor_copy(Wb_sb[:], W_sb[:])

            # ---- phase A: load shard, row norms, colsum(xn) partial ----
            for i in range(T):
                xs = x_all[:, i * D:(i + 1) * D]
                nc.sync.dma_start(xs, x[i * P:(i + 1) * P, :])
                scr = wp.tile([P, D], F32, tag="scr", name=f"scr{i}")
                nc.scalar.activation(scr[:], xs, AF.Square, accum_out=ss[:, i:i + 1])
                nc.vector.tensor_copy(xb_all[:, i * D:(i + 1) * D], xs)
            nc.scalar.activation(nrm[:], ss[:], AF.Sqrt)
            nc.vector.reciprocal(invn[:], nrm[:])

            psum_t = psp.tile([1, D], F32, name="psum_t")
            for i in range(T):
                nc.tensor.matmul(
                    psum_t[:], lhsT=invn[:, i:i + 1], rhs=x_all[:, i * D:(i + 1) * D],
                    start=(i == 0), stop=(i == T - 1),
                )
            t_sb = pp.tile([1, D], F32)
            nc.vector.tensor_copy(t_sb[:], psum_t[:])
            nc.sync.dma_start(cc_t_in[:], t_sb[:])
            nc.gpsimd.collective_compute(
                "AllGather", ALU.bypass, replica_groups=rg,
                ins=[cc_t_in.opt()], outs=[cc_t_out.opt()],
            )

            # x.T via PE transposes (independent of the collective -> overlaps it)
            for i in range(T):
                for c in range(2):
                    pt = psw.tile([P, P], F32, tag="pw", name=f"pt{i}_{c}")
                    nc.tensor.transpose(
                        pt[:], x_all[:, i * D + c * P: i * D + (c + 1) * P], ident[:]
                    )
                    nc.vector.tensor_copy(xT_all[:, c * NL + i * P: c * NL + (i + 1) * P], pt[:])

            tg_sb = pp.tile([8, D], F32)
            nc.sync.dma_start(tg_sb[:], cc_t_out[:])
            # sum the 8 rank partials AND broadcast to 128 partitions in one matmul
            psum_tb = psp.tile([P, D], F32, name="psum_tb")
            nc.tensor.matmul(psum_tb[:], lhsT=ones8[:], rhs=tg_sb[:], start=True, stop=True)

            # ---- phase B: degrees, f, g = f*x, zT partial, zw partial ----
            tb_sb = pp.tile([P, D], F32)
            nc.vector.tensor_copy(tb_sb[:], psum_tb[:])
            big_scr = pp.tile([P, T * D], F32)
            t_ap = tb_sb[:]
            from concourse.bass_types import AP as _AP
            t_rep = _AP(t_ap.tensor, t_ap.offset, [t_ap.ap[0], [0, T], t_ap.ap[1]])
            x3 = x_all[:].rearrange("p (t d) -> p t d", t=T)
            s3 = big_scr[:].rearrange("p (t d) -> p t d", t=T)
            nc.vector.tensor_mul(s3, x3, t_rep)
            nc.vector.tensor_reduce(stl[:], s3, axis=mybir.AxisListType.X, op=ALU.add)
            nc.vector.tensor_mul(s_t[:], stl[:], invn[:])       # s = rowsum * invn
            nc.scalar.activation(sq_s[:], s_t[:], AF.Sqrt)
            nc.vector.reciprocal(dd[:], sq_s[:])                # d = rsqrt(s)
            nc.vector.tensor_mul(f_t[:], dd[:], invn[:])        # f = d * invn
            for i in range(T):
                nc.scalar.mul(g_all[:, i * D:(i + 1) * D], x_all[:, i * D:(i + 1) * D],
                              f_t[:, i:i + 1])

            psum_zT0 = psp.tile([P, D], F32, name="pzT0")
            psum_zT1 = psp.tile([P, D], F32, name="pzT1")
            for i in range(T):
                for c, pz in ((0, psum_zT0), (1, psum_zT1)):
                    nc.tensor.matmul(
                        pz[:], lhsT=xb_all[:, i * D + c * P: i * D + (c + 1) * P],
                        rhs=g_all[:, i * D:(i + 1) * D],
                        start=(i == 0), stop=(i == T - 1),
                    )
            for c, pz in ((0, psum_zT0), (1, psum_zT1)):
                nc.vector.tensor_copy(zT_sb[:, c * D:(c + 1) * D], pz[:])


            # zw partial = z_p @ W (fold the W GEMM before the collective)
            for m in range(2):
                pzw = psw.tile([P, D], F32, tag="pw", name=f"pzw{m}")
                for kc in range(2):
                    nc.tensor.matmul(
                        pzw[:], lhsT=zT_sb[:, kc * D + m * P: kc * D + (m + 1) * P],
                        rhs=Wb_sb[:, kc * D:(kc + 1) * D],
                        start=(kc == 0), stop=(kc == 1),
                    )
                zwp_sb = wp.tile([P, D], BF16, tag="zwp", name=f"zwp{m}")
                nc.vector.tensor_copy(zwp_sb[:], pzw[:])
                nc.sync.dma_start(cc_zw_in[m * P:(m + 1) * P, :], zwp_sb[:])
            nc.gpsimd.collective_compute(
                "AllReduce", ALU.add, replica_groups=rg,
                ins=[cc_zw_in.opt()], outs=[cc_zw_out.opt()],
            )
            for ka in range(2):
                nc.sync.dma_start(zw_sb[:, ka * D:(ka + 1) * D], cc_zw_out[ka * P:(ka + 1) * P, :])

            # ---- phase C: out = f * (x @ zw) ----
            for i in range(T):
                po = psw.tile([P, D], F32, tag="pw", name=f"po{i}")
                for ka in range(2):
                    nc.tensor.matmul(
                        po[:], lhsT=xT_all[:, ka * NL + i * P: ka * NL + (i + 1) * P],
                        rhs=zw_sb[:, ka * D:(ka + 1) * D],
                        start=(ka == 0), stop=(ka == 1),
                    )
                o_sb = wp.tile([P, D], F32, tag="osb", name=f"osb{i}")
                nc.scalar.mul(o_sb[:], po[:], f_t[:, i:i + 1])
                nc.sync.dma_start(out[i * P:(i + 1) * P, :], o_sb[:])


def _build():
    nc = bacc.Bacc("TRN2", target_bir_lowering=False, debug=False, num_devices=R)
    x = nc.dram_tensor("x", [NL, D], F32, kind="ExternalInput")
    W = nc.dram_tensor("W", [D, D], F32, kind="ExternalInput")
    out = nc.dram_tensor("out", [NL, D], F32, kind="ExternalOutput")
    with tile.TileContext(nc) as tc:
        _program(tc, x.ap() if hasattr(x, "ap") else x, W.ap() if hasattr(W, "ap") else W, out.ap() if hasattr(out, "ap") else out)
    nc.finalize()
    return nc


def _run(inputs, trace=False):
    if "nc" not in _cache:
        _cache["nc"] = _build()
    nc = _cache["nc"]
    x = np.ascontiguousarray(inputs["x"], dtype=np.float32)
    W = np.ascontiguousarray(inputs["W"], dtype=np.float32)
    in_maps = [{"x": x[r * NL:(r + 1) * NL], "W": W} for r in range(R)]
    res = bass_utils.run_bass_kernel_spmd(
        nc, in_maps, core_ids=list(range(R)), trace=trace,
    )
    out = np.concatenate([res.results[r]["out"] for r in range(R)], axis=0)
    return out, res


def kernel(**inputs) -> np.ndarray:
    out, _ = _run(inputs, trace=False)
    return out

